# revision 1
# baseline (speedup 1.0000x reference)
"""Multi-head causal attention (B=2, S=2048, D=1024, H=16) on 8 trn2 NeuronCores.

Sharding: 2 heads per core (tensor-parallel over the 16 heads). Each core
receives the full (host-pre-transposed, fp16) activations plus its own slice
of the projection weights, computes

    qhT/khT = (Wq_l @ x.T)      [128, B*S]   (head dim on partitions)
    vh      = x @ Wv_l.T        per 128-row chunk, stored [j, c] + ones column
    S_T     = kh @ qh.T / 8     per (b, h-pair), keys on partitions
    P       = exp(S_T + pad_bias) * causal_mask   (mask only on the 128-wide
                                                   diagonal sub-block)
    attT    = (v_aug.T @ P) -> rows 0:64 = att.T, row 64 = softmax denominator
    aT      = attT * (1/denom)  (denominator broadcast onto 64 partitions via a
              ones-column PE matmul; reciprocal via the fast DVE approx)
    y_part  = A_local @ Wo[:, mslice].T     [B*S, D] partial sums

The host sums the 8 partial outputs (standard row-parallel unshard) and
reshapes to [B, S, D] float32.

Scheduling: phases are issued as A(b0) -> B(b0)+C(b0) -> A(b1) -> B(b1)+C(b1);
the Tile list scheduler overlaps B's scalar-engine exp work with the next
batch's projection matmuls, keeping the PE dense (and HAM-warm).
"""

import os
import sys

import numpy as np

try:
    import concourse.bass as bass
except ImportError:  # fallback if sitecustomize did not add the repo
    for _p in ("/opt/trn_rl_repo", "/root/.axon_site/_ro/trn_rl_repo"):
        if os.path.isdir(_p) and _p not in sys.path:
            sys.path.insert(0, _p)
    import concourse.bass as bass  # noqa: F401

import concourse.tile as tile
from concourse import bacc, mybir
from concourse.bass_utils import run_bass_kernel_spmd

B, S, D, H, DK = 2, 2048, 1024, 16, 64
BS = B * S                # 4096
N_CORES = 8
H_LOC = H // N_CORES      # 2 heads per core
M_LOC = H_LOC * DK        # 128 local concat dim
NJT = S // 128            # 16 key tiles per batch

F16 = mybir.dt.float16
F32 = mybir.dt.float32
FT = mybir.ActivationFunctionType

_CACHE = {}


def _build_nc(reps=1):
    """Build + compile the per-core Bass program (identical across cores).

    reps > 1 repeats the whole (idempotent) body back-to-back inside one
    NEFF — used by test.py to time one rep via the slope between NEFFs.
    """
    from contextlib import ExitStack

    nc = bacc.Bacc(
        "TRN2", target_bir_lowering=False, debug=False, enable_asserts=False
    )

    io = {}
    for nm in ("xqT", "xkT", "xvT"):
        io[nm] = nc.dram_tensor(nm, [D, BS], F16, kind="ExternalInput").ap()
    # packed consts: one trigger each for the three projection weights and for
    # woT+causal-mask (every serial ~0.6us sync trigger before the first x
    # load delays the first matmul)
    io["wqkv"] = nc.dram_tensor(
        "wqkv", [128, 3, 8, 128], F16, kind="ExternalInput"
    ).ap()
    io["wo_cm"] = nc.dram_tensor(
        "wo_cm", [128, D + H_LOC * 128], F16, kind="ExternalInput"
    ).ap()
    io["pad_bias"] = nc.dram_tensor(
        "pad_bias", [128, B, NJT], F32, kind="ExternalInput"
    ).ap()
    io["y"] = nc.dram_tensor("y_partial", [BS, D], F16, kind="ExternalOutput").ap()

    with tile.TileContext(nc) as tc, ExitStack() as ctx:
        pools = {
            "const": ctx.enter_context(tc.tile_pool(name="const", bufs=1)),
            "xpool": ctx.enter_context(tc.tile_pool(name="xpool", bufs=24)),
            "ppool": ctx.enter_context(tc.tile_pool(name="ppool", bufs=6)),
            "mpool": ctx.enter_context(tc.tile_pool(name="mpool", bufs=2)),
            "apool": ctx.enter_context(tc.tile_pool(name="apool", bufs=2)),
            "ypool": ctx.enter_context(tc.tile_pool(name="ypool", bufs=3)),
            # PSUM budget (8 banks): ps 2x2 + po 1x2 + mm 2x1 = 8
            "psum_s": ctx.enter_context(
                tc.tile_pool(name="psum_s", bufs=2, space="PSUM")
            ),
            "psum_o": ctx.enter_context(
                tc.tile_pool(name="psum_o", bufs=1, space="PSUM")
            ),
            "psum_mm": ctx.enter_context(
                tc.tile_pool(name="psum_mm", bufs=2, space="PSUM")
            ),
        }
        for _ in range(reps):
            _body(pools, tc, io)
    nc.compile()
    return nc


def _body(pools, tc, io):
    nc = tc.nc
    PH = os.environ.get("KERNEL_PHASES", "ABC")  # debug: subset of phases

    const = pools["const"]
    xpool = pools["xpool"]
    ppool = pools["ppool"]
    mpool = pools["mpool"]
    apool = pools["apool"]
    ypool = pools["ypool"]
    psum_s = pools["psum_s"]
    psum_o = pools["psum_o"]
    psum_mm = pools["psum_mm"]

    # ---- constants / persistent buffers ----
    wqkv_sb = const.tile([128, 3, 8, 128], F16, name="wqkv_sb")
    nc.sync.dma_start(out=wqkv_sb, in_=io["wqkv"])
    w_sb = {nm: wqkv_sb[:, i] for i, nm in enumerate(("wq", "wk", "wv"))}
    wo_cm_sb = const.tile([128, D + H_LOC * 128], F16, name="wo_cm_sb")
    nc.sync.dma_start(out=wo_cm_sb, in_=io["wo_cm"])
    woT_sb = wo_cm_sb[:, 0:D]
    cmask_sb = wo_cm_sb[:, D : D + H_LOC * 128].rearrange(
        "p (h i) -> p h i", h=H_LOC
    )
    pbias_sb = const.tile([128, B, NJT], F32, name="pbias_sb")
    nc.sync.dma_start(out=pbias_sb, in_=io["pad_bias"])
    ones_sb = const.tile([128, DK], F16, name="ones_sb")
    nc.vector.memset(ones_sb, 1.0)

    qhT_sb = const.tile([128, BS], F16, name="qhT_sb")  # [c2, b*S + i]
    khT_sb = const.tile([128, BS], F16, name="khT_sb")
    vaug_sb = const.tile([128, B, H_LOC, NJT, DK + 1], F16, name="vaug_sb")
    nc.vector.memset(vaug_sb[:, :, :, :, DK : DK + 1], 1.0)
    aT_sb = const.tile([128, BS], F16, name="aT_sb")

    # ---- phase A: projections for one 1024-col seq supertile ----
    def phase_a(ss, inject=None):
        for nm in ("q", "k", "v"):
            if nm == "k" and inject is not None:
                inject()  # e.g. the previous batch's trailing phase-C chunk
            xT = io["x" + nm + "T"]
            xts = []
            for kk in range(8):
                xt = xpool.tile([128, 1024], F16, name=f"x{nm}_{ss}_{kk}", tag="xt")
                nc.sync.dma_start(
                    out=xt,
                    in_=xT[kk * 128 : (kk + 1) * 128, ss * 1024 : (ss + 1) * 1024],
                )
                xts.append(xt)
            if nm != "v":
                outbuf = {"q": qhT_sb, "k": khT_sb}[nm]
                for sc in range(2):
                    ps = psum_mm.tile(
                        [128, 512], F32, name=f"psA{nm}_{ss}_{sc}", tag="mm"
                    )
                    for kk in range(8):
                        nc.tensor.matmul(
                            ps,
                            lhsT=w_sb["w" + nm][:, kk, :],
                            rhs=xts[kk][:, sc * 512 : (sc + 1) * 512],
                            start=(kk == 0),
                            stop=(kk == 7),
                        )
                    col = ss * 1024 + sc * 512
                    nc.any.tensor_copy(outbuf[:, col : col + 512], ps)
            else:
                for sp in range(2):  # four 128-row chunks -> one PSUM bank
                    ps = psum_mm.tile(
                        [128, 4, 128], F32, name=f"psV_{ss}_{sp}", tag="mm"
                    )
                    for i4 in range(4):
                        sl = sp * 4 + i4
                        for kk in range(8):
                            nc.tensor.matmul(
                                ps[:, i4, :],
                                lhsT=xts[kk][:, sl * 128 : (sl + 1) * 128],
                                rhs=w_sb["wv"][:, kk, :],
                                start=(kk == 0),
                                stop=(kk == 7),
                            )
                    sch0 = ss * 8 + sp * 4
                    b, jt0 = divmod(sch0, NJT)
                    nc.any.tensor_copy(
                        vaug_sb[:, b, :, jt0 : jt0 + 4, 0:DK].rearrange(
                            "p h j c -> p j h c"
                        ),
                        ps.rearrange("p j (h c) -> p j h c", h=H_LOC),
                    )

    # ---- phase B: attention; divide via PE ones-broadcast + fast reciprocal

    def divide(b, ic, araw):
        """aT cols [ic*512,(ic+1)*512) = araw rows / denominator row (row 64)."""
        dcols = slice(b * S + ic * 512, b * S + (ic + 1) * 512)
        for h in range(H_LOC):
            bch = psum_mm.tile([DK, 512], F32, name=f"bc_{b}{ic}{h}", tag="mm")
            nc.tensor.matmul(
                bch,
                lhsT=ones_sb[DK : DK + 1, :],
                rhs=araw[DK : DK + 1, h, :],
                start=True,
                stop=True,
            )
            rcph = mpool.tile([DK, 512], F32, name=f"rc_{b}{ic}{h}", tag="rcp")
            nc.vector.reciprocal_approx_fast(rcph, bch)
            if h == 0:
                nc.vector.tensor_mul(aT_sb[0:DK, dcols], araw[0:DK, 0, :], rcph)
            else:
                tmpa = mpool.tile([DK, 512], F16, name=f"ta_{b}{ic}", tag="ta")
                nc.vector.tensor_mul(tmpa, araw[0:DK, 1, :], rcph)
                # partition remap 0:64 -> 64:128 via SBUF->SBUF DMA
                nc.sync.dma_start(out=aT_sb[DK : 2 * DK, dcols], in_=tmpa)

    def phase_c(b, ic):
        for sch in range(ic * 4, ic * 4 + 4):
            ysb = ypool.tile([128, 1024], F16, name=f"ysb_{b}{sch}", tag="ysb")
            for eh in range(2):
                py = psum_mm.tile([128, 512], F32, name=f"psC_{b}{sch}{eh}", tag="mm")
                nc.tensor.matmul(
                    py,
                    lhsT=aT_sb[:, b * S + sch * 128 : b * S + (sch + 1) * 128],
                    rhs=woT_sb[:, eh * 512 : (eh + 1) * 512],
                    start=True,
                    stop=True,
                )
                nc.any.tensor_copy(ysb[:, eh * 512 : (eh + 1) * 512], py)
            r0 = b * S + sch * 128
            nc.sync.dma_start(out=io["y"][r0 : r0 + 128, :], in_=ysb)

    def phase_b(b):
        # phase C for chunk ic-1 is issued AFTER the first few score tiles of
        # chunk ic: C's matmuls read aT rows 64:128 (the h1 remap DMA), and
        # issuing them immediately after divide() puts them ahead of the next
        # chunk's (always-ready) score matmuls in the static PE order — a
        # measured ~4-6us head-of-line stall per chunk boundary.
        pending_c = None
        for ic in range(4):  # query chunks of 512
            njt = 4 * (ic + 1)  # causal: keys up to end of this query chunk
            po = psum_o.tile([DK + 1, H_LOC, 512], F32, name=f"po_{b}{ic}", tag="po")
            for jt in range(njt):
                # deep enough that the previous chunk's h1 remap DMA (which
                # C's matmuls read) has completed by the time C hits the PE
                # queue head
                cjt = min(6, njt - 2)
                if jt == cjt and pending_c is not None and "C" in PH:
                    phase_c(b, pending_c)
                    pending_c = None
                jcols = slice(b * S + jt * 128, b * S + (jt + 1) * 128)
                # diagonal tiles: columns i < (jt-4ic)*128 are fully masked --
                # skip them in the scores matmul, exp, mask, and PV.
                o = jt - 4 * ic
                lo = o * 128 if o >= 0 else 0
                ps = psum_s.tile(
                    [128, H_LOC, 512], F32, name=f"psS_{b}{ic}{jt}", tag="ps"
                )
                for h in range(H_LOC):
                    r0 = DK * h
                    nc.tensor.matmul(
                        ps[:, h, lo:512],
                        lhsT=khT_sb[r0 : r0 + DK, jcols],
                        rhs=qhT_sb[
                            r0 : r0 + DK, b * S + ic * 512 + lo : b * S + (ic + 1) * 512
                        ],
                        start=True,
                        stop=True,
                    )
                pe = ppool.tile(
                    [128, H_LOC, 512], F16, name=f"pe_{b}{ic}{jt}", tag="pe"
                )
                nc.scalar.activation(
                    pe[:, :, lo:512],
                    ps[:, :, lo:512],
                    FT.Exp,
                    bias=pbias_sb[:, b, jt : jt + 1],
                    scale=0.125,
                )
                if o >= 0:  # causal zero-mask on the 128-wide diagonal block
                    nc.vector.tensor_mul(
                        pe[:, :, lo : lo + 128], pe[:, :, lo : lo + 128], cmask_sb
                    )
                for h in range(H_LOC):
                    nc.tensor.matmul(
                        po[:, h, lo:512],
                        lhsT=vaug_sb[:, b, h, jt, :],
                        rhs=pe[:, h, lo:512],
                        start=(jt == 0),
                        stop=(jt == njt - 1),
                    )
            # one quick copy releases po; pinned to the vector engine so it
            # can't be queued on ACT behind the next chunk's exp work (po
            # release gates the next chunk's PV accumulation)
            araw = apool.tile(
                [DK + 1, H_LOC, 512], F16, name=f"araw_{b}{ic}", tag="ar"
            )
            nc.vector.tensor_copy(araw, po)
            divide(b, ic, araw)
            pending_c = ic
        return pending_c

    if "A" in PH:
        phase_a(0)
        phase_a(1)
    pc0 = None
    if "B" in PH:
        pc0 = phase_b(0)
    if "A" in PH:
        inj = (lambda: phase_c(0, pc0)) if (pc0 is not None and "C" in PH) else None
        phase_a(2, inject=inj)
        phase_a(3)
    elif pc0 is not None and "C" in PH:
        phase_c(0, pc0)
    if "B" in PH:
        pc1 = phase_b(1)
        if pc1 is not None and "C" in PH:
            phase_c(1, pc1)


def get_nc():
    if "nc" not in _CACHE:
        _CACHE["nc"] = _build_nc()
    return _CACHE["nc"]


def prep_inputs(q, k, v, mask, Wq, Wk, Wv, Wo):
    """Host-side shard prep: transposes, fp16 casts, per-core weight slices."""
    q = np.asarray(q, dtype=np.float32).reshape(BS, D)
    k = np.asarray(k, dtype=np.float32).reshape(BS, D)
    v = np.asarray(v, dtype=np.float32).reshape(BS, D)
    mask = np.asarray(mask)
    Wq, Wk, Wv, Wo = (np.asarray(w, dtype=np.float32) for w in (Wq, Wk, Wv, Wo))

    xqT = np.ascontiguousarray(q.T).astype(np.float16)
    xkT = np.ascontiguousarray(k.T).astype(np.float16)
    xvT = np.ascontiguousarray(v.T).astype(np.float16)

    pb = np.where(mask == 0, np.float32(-1e9), np.float32(0.0)).astype(np.float32)
    # [B, S] -> [128, B, S//128]  (partition = j % 128, col = key tile)
    pad_bias = np.ascontiguousarray(pb.reshape(B, S // 128, 128).transpose(2, 0, 1))

    # [128, H_LOC, 128] lower-triangle mask for the diagonal sub-block:
    # within the block, token col i is live for key row p iff i >= p.
    p_idx = np.arange(128)[:, None]
    i_idx = np.arange(128)[None, :]
    cm = (i_idx >= p_idx).astype(np.float16)  # [128, 128]
    cmask = np.ascontiguousarray(
        np.broadcast_to(cm[:, None, :], (128, H_LOC, 128))
    )

    def wslice(Wmat, c):
        ws = Wmat[c * M_LOC : (c + 1) * M_LOC, :]  # [128 out, 1024 in]
        # -> [p(=d%128), kk(=d//128), c2]
        return np.ascontiguousarray(
            ws.T.reshape(8, 128, M_LOC).transpose(1, 0, 2)
        ).astype(np.float16)

    cmask_flat = cmask.reshape(128, H_LOC * 128)
    in_maps = []
    for c in range(N_CORES):
        woT_c = np.ascontiguousarray(Wo[:, c * M_LOC : (c + 1) * M_LOC].T).astype(
            np.float16
        )
        wqkv = np.ascontiguousarray(
            np.stack([wslice(Wq, c), wslice(Wk, c), wslice(Wv, c)], axis=1)
        )
        wo_cm = np.ascontiguousarray(
            np.concatenate([woT_c, cmask_flat], axis=1)
        )
        in_maps.append(
            {
                "xqT": xqT,
                "xkT": xkT,
                "xvT": xvT,
                "wqkv": wqkv,
                "wo_cm": wo_cm,
                "pad_bias": pad_bias,
            }
        )
    return in_maps


def gather_output(results):
    acc = np.zeros((BS, D), dtype=np.float32)
    for r in results:
        acc += r["y_partial"].astype(np.float32)
    return acc.reshape(B, S, D)


def kernel(q, k, v, mask, Wq, Wk, Wv, Wo):
    nc = get_nc()
    in_maps = prep_inputs(q, k, v, mask, Wq, Wk, Wv, Wo)
    res = run_bass_kernel_spmd(nc, in_maps, core_ids=list(range(N_CORES)))
    return gather_output(res.results)



# revision 25
# speedup vs baseline: 1.0552x; 1.0552x over previous
"""Multi-head causal attention (B=2, S=2048, D=1024, H=16) on 8 trn2 NeuronCores.

Sharding: 2 heads per core (tensor-parallel over the 16 heads). Each core
receives the full (host-pre-transposed, fp16) activations plus its own slice
of the projection weights, computes

    qhT/khT = (Wq_l @ x.T)      [128, B*S]   (head dim on partitions)
    vh      = x @ Wv_l.T        per 128-row chunk, stored [j, c] + ones column
    S_T     = kh @ qh.T / 8     per (b, h-pair), keys on partitions
    P       = exp(S_T + pad_bias) * causal_mask   (mask only on the 128-wide
                                                   diagonal sub-block)
    attT    = (v_aug.T @ P) -> rows 0:64 = att.T, row 64 = softmax denominator
    aT      = attT * (1/denom)  (denominator broadcast onto 64 partitions via a
              ones-column PE matmul; reciprocal via the fast DVE approx)
    y_part  = A_local @ Wo[:, mslice].T     [B*S, D] partial sums

The host sums the 8 partial outputs (standard row-parallel unshard) and
reshapes to [B, S, D] float32.

Scheduling (v2): the whole kernel is emitted as one interleaved stream aimed
at keeping the PE continuously busy (which also keeps it at max p-state):

  - phase B is software-pipelined: the PE order per key tile is
    S(t) ... S(t+1), [filler], PV(t), so PV never head-of-line blocks the
    in-order PE queue while the scalar engine is still computing exp(t).
  - phase A (projection) and phase C (output proj) work is chopped into
    ~0.5-1.7us quanta and injected as the [filler] between B tiles, ordered
    against DMA arrival of the x supertiles.
  - engine assignment: ACT = exp only (+ some C drains), DVE = PSUM drains +
    reciprocal, GpSimd/Pool = causal-mask mul + divide mul + y/remap DMA
    triggers, so no engine exceeds the PE's ~116us of streaming work.
"""

import os
import sys
from collections import deque

import numpy as np

try:
    import concourse.bass as bass
except ImportError:  # fallback if sitecustomize did not add the repo
    for _p in ("/opt/trn_rl_repo", "/root/.axon_site/_ro/trn_rl_repo"):
        if os.path.isdir(_p) and _p not in sys.path:
            sys.path.insert(0, _p)
    import concourse.bass as bass  # noqa: F401

import concourse.tile as tile
from concourse import bacc, mybir
from concourse.bass_utils import run_bass_kernel_spmd

B, S, D, H, DK = 2, 2048, 1024, 16, 64
BS = B * S                # 4096
N_CORES = 8
H_LOC = H // N_CORES      # 2 heads per core
M_LOC = H_LOC * DK        # 128 local concat dim
NJT = S // 128            # 16 key tiles per batch

F16 = mybir.dt.float16
F32 = mybir.dt.float32
FT = mybir.ActivationFunctionType

_CACHE = {}


def _build_nc(reps=1):
    """Build + compile the per-core Bass program (identical across cores)."""
    from contextlib import ExitStack

    nc = bacc.Bacc(
        "TRN2", target_bir_lowering=False, debug=False, enable_asserts=False
    )

    io = {}
    for nm in ("xqT", "xkT", "xvT"):
        io[nm] = nc.dram_tensor(nm, [D, BS], F16, kind="ExternalInput").ap()
    io["wqkv"] = nc.dram_tensor(
        "wqkv", [128, 3, 8, 128], F16, kind="ExternalInput"
    ).ap()
    io["wo_cm"] = nc.dram_tensor(
        "wo_cm", [128, D + H_LOC * 128], F16, kind="ExternalInput"
    ).ap()
    io["pad_bias"] = nc.dram_tensor(
        "pad_bias", [128, B, NJT], F32, kind="ExternalInput"
    ).ap()
    io["y"] = nc.dram_tensor("y_partial", [BS, D], F16, kind="ExternalOutput").ap()

    with tile.TileContext(nc) as tc, ExitStack() as ctx:
        pools = {
            "const": ctx.enter_context(tc.tile_pool(name="const", bufs=1)),
            "xpool": ctx.enter_context(tc.tile_pool(name="xpool", bufs=40)),
            "ppool": ctx.enter_context(tc.tile_pool(name="ppool", bufs=6)),
            "mpool": ctx.enter_context(tc.tile_pool(name="mpool", bufs=3)),
            "apool": ctx.enter_context(tc.tile_pool(name="apool", bufs=2)),
            "ypool": ctx.enter_context(tc.tile_pool(name="ypool", bufs=4)),
            # PSUM budget (8 banks): ps 2x2 + po 1x2 + mm 2x1 = 8
            "psum_s": ctx.enter_context(
                tc.tile_pool(name="psum_s", bufs=2, space="PSUM")
            ),
            "psum_o": ctx.enter_context(
                tc.tile_pool(name="psum_o", bufs=1, space="PSUM")
            ),
            "psum_mm": ctx.enter_context(
                tc.tile_pool(name="psum_mm", bufs=2, space="PSUM")
            ),
        }
        for _ in range(reps):
            _body(pools, tc, io)
    nc.compile()
    return nc


def _body(pools, tc, io):
    nc = tc.nc

    const = pools["const"]
    xpool = pools["xpool"]
    ppool = pools["ppool"]
    mpool = pools["mpool"]
    apool = pools["apool"]
    ypool = pools["ypool"]
    psum_s = pools["psum_s"]
    psum_o = pools["psum_o"]
    psum_mm = pools["psum_mm"]

    # ---- constants / persistent buffers ----
    # wqkv is split per tensor (wq first) so the very first projection
    # matmuls are not gated on the whole 786KB constant transfer.
    wqkv_sb = const.tile([128, 3, 8, 128], F16, name="wqkv_sb")
    w_sb = {nm: wqkv_sb[:, i] for i, nm in enumerate(("wq", "wk", "wv"))}
    wo_cm_sb = const.tile([128, D + H_LOC * 128], F16, name="wo_cm_sb")
    woT_sb = wo_cm_sb[:, 0:D]
    cmask_sb = wo_cm_sb[:, D : D + H_LOC * 128].rearrange(
        "p (h i) -> p h i", h=H_LOC
    )
    pbias_sb = const.tile([128, B, NJT], F32, name="pbias_sb")
    ones_sb = const.tile([128, DK], F16, name="ones_sb")
    nc.vector.memset(ones_sb, 1.0)

    qhT_sb = const.tile([128, BS], F16, name="qhT_sb")  # [c2, b*S + i]
    khT_sb = const.tile([128, BS], F16, name="khT_sb")
    vaug_sb = const.tile([128, B, H_LOC, NJT, DK + 1], F16, name="vaug_sb")
    nc.vector.memset(vaug_sb[:, :, :, :, DK : DK + 1], 1.0)
    aT_sb = const.tile([128, BS], F16, name="aT_sb")

    # ---- x piece DMA: all triggers issued up front on the sync sequencer
    # in consumption-priority order (the DMA rings process descriptors in
    # trigger order, so priority = issue order). xpool bufs gate the later
    # triggers so this is a deep rolling prefetch. Compute engines must NOT
    # trigger these (an in-order compute queue blocked on a buf-gated
    # trigger would stall its real work).
    xpiece = {}

    def load_x(nm, ss, half=None):
        """half=None: one [128,1024] DMA per piece; half=0/1: load only that
        512-col half (the ss0 pieces are split so the first score tiles are
        not gated on the full first supertile)."""
        xT = io["x" + nm + "T"]
        c0 = 0 if half is None else half * 512
        cw = 1024 if half is None else 512
        for kk in range(8):
            key = (nm, ss, kk)
            if key not in xpiece:
                xpiece[key] = xpool.tile(
                    [128, 1024], F16, name=f"x{nm}{ss}_{kk}", tag="xt"
                )
            nc.sync.dma_start(
                out=xpiece[key][:, c0 : c0 + cw],
                in_=xT[
                    kk * 128 : (kk + 1) * 128,
                    ss * 1024 + c0 : ss * 1024 + c0 + cw,
                ],
            )

    nc.sync.dma_start(out=wqkv_sb[:, 0], in_=io["wqkv"][:, 0])
    load_x("q", 0)
    nc.sync.dma_start(out=wqkv_sb[:, 1], in_=io["wqkv"][:, 1])
    load_x("k", 0)
    nc.sync.dma_start(out=wqkv_sb[:, 2], in_=io["wqkv"][:, 2])
    load_x("v", 0)
    nc.sync.dma_start(out=wo_cm_sb, in_=io["wo_cm"])
    nc.sync.dma_start(out=pbias_sb, in_=io["pad_bias"])
    for ss in range(1, 4):
        load_x("q", ss)
        load_x("k", ss)
        load_x("v", ss)

    # ---- quantum emitters ----
    drain_flip = [0]

    def emit_aqk(nm, ss, sc):
        """q/k projection for one 512-col unit: 8 matmuls + 1 DVE drain."""
        outbuf = {"q": qhT_sb, "k": khT_sb}[nm]
        ps = psum_mm.tile([128, 512], F32, name=f"psA{nm}{ss}{sc}", tag="mm")
        for kk in range(8):
            nc.tensor.matmul(
                ps,
                lhsT=w_sb["w" + nm][:, kk, :],
                rhs=xpiece[(nm, ss, kk)][:, sc * 512 : (sc + 1) * 512],
                start=(kk == 0),
                stop=(kk == 7),
            )
        col = ss * 1024 + sc * 512
        nc.vector.tensor_copy(outbuf[:, col : col + 512], ps)

    def emit_av(ss, sp):
        """v projection for 4 token blocks (one 512-col half-supertile)."""
        ps = psum_mm.tile([128, 4, 128], F32, name=f"psV{ss}{sp}", tag="mm")
        for i4 in range(4):
            sl = sp * 4 + i4
            for kk in range(8):
                nc.tensor.matmul(
                    ps[:, i4, :],
                    lhsT=xpiece[("v", ss, kk)][:, sl * 128 : (sl + 1) * 128],
                    rhs=w_sb["wv"][:, kk, :],
                    start=(kk == 0),
                    stop=(kk == 7),
                )
        sch0 = ss * 8 + sp * 4
        b, jt0 = divmod(sch0, NJT)
        nc.vector.tensor_copy(
            vaug_sb[:, b, :, jt0 : jt0 + 4, 0:DK].rearrange("p h j c -> p j h c"),
            ps.rearrange("p j (h c) -> p j h c", h=H_LOC),
        )

    def emit_c(b, ic, q0, sch, tail=False):
        """output projection for one 128-token block: 2 matmuls + drains.

        y store DMA is triggered from the gpsimd sequencer so it cannot
        block behind the x-load triggers queued on sync.
        """
        ensure_epi(b, ic, q0)  # aT cols for this block must be emitted first
        ysb = ypool.tile([128, 1024], F16, name=f"ysb_{b}{sch}", tag="ysb")
        r0 = b * S + sch * 128
        for eh in range(2):
            py = psum_mm.tile([128, 512], F32, name=f"psC{b}{sch}{eh}", tag="mm")
            nc.tensor.matmul(
                py,
                lhsT=aT_sb[:, b * S + sch * 128 : b * S + (sch + 1) * 128],
                rhs=woT_sb[:, eh * 512 : (eh + 1) * 512],
                start=True,
                stop=True,
            )
            # PSUM drain engine: 1-in-3 on ACT while b0's exp load is light,
            # all-DVE during b1 (ACT saturates on exp late in the kernel),
            # ACT+DVE in parallel for the tail blocks (exp is finished)
            if tail:
                eng_copy = nc.scalar.copy if eh == 0 else nc.vector.tensor_copy
                eng_copy(ysb[:, eh * 512 : (eh + 1) * 512], py)
            elif b == 0 and drain_flip[0] % 3 == 0:
                nc.scalar.copy(ysb[:, eh * 512 : (eh + 1) * 512], py)
            else:
                nc.vector.tensor_copy(ysb[:, eh * 512 : (eh + 1) * 512], py)
            drain_flip[0] += 1
            # store each 512-col half separately for more DMA ring spread
            nc.sync.dma_start(
                out=io["y"][r0 : r0 + 128, eh * 512 : (eh + 1) * 512],
                in_=ysb[:, eh * 512 : (eh + 1) * 512],
            )

    # ---- filler queue ----
    # entries are (kind, fn). 'a' quanta (projections) inject every tile:
    # they gate later phases and are paced by the x DMA anyway. 'c' quanta
    # have loose deadlines and inject every other tile, so the filler pool
    # stretches over the whole of phase B instead of running dry early.
    filler = deque()
    tile_ctr = [0]

    def inject():
        tile_ctr[0] += 1
        if not filler:
            return
        kind, fn = filler[0]
        if kind == "c" and tile_ctr[0] % 2 == 0:
            return
        filler.popleft()
        fn()

    def queue_c(b, ic, q0, qw, tail=False):
        for sch in range((ic * 512 + q0) // 128, (ic * 512 + q0 + qw) // 128):
            filler.append(
                ("c", lambda b=b, ic=ic, q0=q0, sch=sch: emit_c(b, ic, q0, sch, tail))
            )

    # ---- phase B ----
    # The divide part of a chunk's epilogue (PE broadcast -> DVE reciprocal
    # -> Pool mul) depends on the DVE araw drain; emitting it immediately
    # after the last PV would head-of-line stall the in-order PE queue. It
    # is deferred into the next chunk (epi_map), after that chunk's first
    # score tile is already in flight. emit_c calls ensure_epi so a C
    # quantum can never be emitted before the aT writes it reads.
    epi_map = {}
    prev_chunk = [None]

    def ensure_epi(b, ic, q0):
        f = epi_map.get((b, ic, q0))
        if f is not None:
            epi_map[(b, ic, q0)] = None
            f()

    def epi_div(b, ic, q0, qw, araw):
        g0 = b * S + ic * 512 + q0
        dcols = slice(g0, g0 + qw)
        for h in range(H_LOC):
            bch = psum_mm.tile([DK, qw], F32, name=f"bc_{b}{ic}{q0}{h}", tag="mm")
            nc.tensor.matmul(
                bch,
                lhsT=ones_sb[DK : DK + 1, :],
                rhs=araw[DK : DK + 1, h, :],
                start=True,
                stop=True,
            )
            rcph = mpool.tile([DK, qw], F32, name=f"rc_{b}{ic}{q0}{h}", tag="rcp")
            nc.vector.reciprocal_approx_fast(rcph, bch)
            if h == 0:
                nc.vector.tensor_mul(aT_sb[0:DK, dcols], araw[0:DK, 0, :], rcph)
            else:
                tmpa = mpool.tile([DK, qw], F16, name=f"ta_{b}{ic}{q0}", tag="ta")
                nc.vector.tensor_mul(tmpa, araw[0:DK, 1, :], rcph)
                # partition remap 0:64 -> 64:128 via SBUF->SBUF DMA
                nc.sync.dma_start(out=aT_sb[DK : 2 * DK, dcols], in_=tmpa)

    po_holder = [None]

    def phase_b(b, chunks):
        """chunks: list of (ic, q0, qw) query sub-chunks, in order."""
        for ic, q0, qw in chunks:
            cstart = ic * 512 + q0  # batch-relative first query
            njt = (cstart + qw) // 128  # causal: keys up to sub-chunk end
            # one full-width po per ic; a split chunk's sub-chunks write
            # disjoint column slices (keeps each head's accumulation group
            # in its own 2KB PSUM zero-region)
            if q0 == 0:
                po_holder[0] = psum_o.tile(
                    [DK + 1, H_LOC, 512], F32, name=f"po_{b}{ic}", tag="po"
                )
            po = po_holder[0][:, :, q0 : q0 + qw]
            pending_pv = deque()  # two-tile PV lag: exp+mask get extra slack
            for jt in range(njt):
                jcols = slice(b * S + jt * 128, b * S + (jt + 1) * 128)
                # diagonal tiles: query cols < jt*128-cstart are fully masked
                dstart = jt * 128 - cstart
                lo = dstart if dstart >= 0 else 0
                ps = psum_s.tile(
                    [128, H_LOC, qw], F32, name=f"psS_{b}{ic}{q0}{jt}", tag="ps"
                )
                for h in range(H_LOC):
                    r0 = DK * h
                    nc.tensor.matmul(
                        ps[:, h, lo:qw],
                        lhsT=khT_sb[r0 : r0 + DK, jcols],
                        rhs=qhT_sb[
                            r0 : r0 + DK,
                            b * S + cstart + lo : b * S + cstart + qw,
                        ],
                        start=True,
                        stop=True,
                    )
                pe = ppool.tile(
                    [128, H_LOC, qw], F16, name=f"pe_{b}{ic}{q0}{jt}", tag="pe"
                )
                nc.scalar.activation(
                    pe[:, :, lo:qw],
                    ps[:, :, lo:qw],
                    FT.Exp,
                    bias=pbias_sb[:, b, jt : jt + 1],
                    scale=0.125,
                )
                if dstart >= 0:  # causal zero-mask on the diagonal block
                    hi = min(lo + 128, qw)
                    nc.vector.tensor_mul(
                        pe[:, :, lo:hi],
                        pe[:, :, lo:hi],
                        cmask_sb[:, :, 0 : hi - lo],
                    )
                if jt == 1 and prev_chunk[0] is not None:
                    ensure_epi(*prev_chunk[0])
                inject()  # filler quanta between S and PV keep the PE dense
                if len(pending_pv) >= 2:
                    pending_pv.popleft()()
                def pv(jt=jt, lo=lo, pe=pe, last=(jt == njt - 1)):
                    for h in range(H_LOC):
                        nc.tensor.matmul(
                            po[:, h, lo:qw],
                            lhsT=vaug_sb[:, b, h, jt, :],
                            rhs=pe[:, h, lo:qw],
                            start=(jt == 0),
                            stop=last,
                        )
                pending_pv.append(pv)
            while pending_pv:
                pending_pv.popleft()()
            # drain po promptly so the next chunk's PV accumulation can
            # claim the bank; the final sub-chunk drains on ACT (idle by
            # then) to keep the tail chain off the still-busy DVE
            araw = apool.tile(
                [DK + 1, H_LOC, qw], F16, name=f"araw_{b}{ic}{q0}", tag="ar"
            )
            if b == 1 and ic == 3 and q0 + qw == 512:
                nc.scalar.copy(araw, po)
            else:
                nc.vector.tensor_copy(araw, po)
            epi_map[(b, ic, q0)] = (
                lambda b=b, ic=ic, q0=q0, qw=qw, araw=araw:
                epi_div(b, ic, q0, qw, araw)
            )
            prev_chunk[0] = (b, ic, q0)
            tail = b == 1 and ic == 3
            queue_c(b, ic, q0, qw, tail=tail)

    # ---- interleaved program ----
    # minimal b0 start set, then B(0) with A/C fillers, then B(1).
    emit_aqk("q", 0, 0)
    emit_aqk("k", 0, 0)
    emit_av(0, 0)
    filler.extend(
        [
            ("a", lambda: emit_aqk("q", 0, 1)),
            ("a", lambda: emit_aqk("k", 0, 1)),
            ("a", lambda: emit_av(0, 1)),
            ("a", lambda: emit_aqk("q", 1, 0)),
            ("a", lambda: emit_aqk("k", 1, 0)),
            ("a", lambda: emit_av(1, 0)),
            ("a", lambda: emit_aqk("q", 1, 1)),
            ("a", lambda: emit_aqk("k", 1, 1)),
            ("a", lambda: emit_av(1, 1)),
            ("a", lambda: emit_aqk("q", 2, 0)),
            ("a", lambda: emit_aqk("k", 2, 0)),
            ("a", lambda: emit_av(2, 0)),
            ("a", lambda: emit_aqk("q", 2, 1)),
            ("a", lambda: emit_aqk("k", 2, 1)),
            ("a", lambda: emit_aqk("q", 3, 0)),
            ("a", lambda: emit_aqk("k", 3, 0)),
            ("a", lambda: emit_av(2, 1)),
        ]
    )
    phase_b(0, [(0, 0, 512), (1, 0, 512), (2, 0, 512), (3, 0, 512)])
    filler.extend(
        [
            ("a", lambda: emit_aqk("q", 3, 1)),
            ("a", lambda: emit_aqk("k", 3, 1)),
            ("a", lambda: emit_av(3, 0)),
            ("a", lambda: emit_av(3, 1)),
        ]
    )
    # C quanta for b0 chunks queued by phase_b(0) sit behind these A tails;
    # rotate them so the b1 A-units run first (they gate B(1) chunks).
    filler.rotate(4)
    # b1's last 512-query chunk is split into 256/128/128-query sub-chunks
    # so the earlier halves' epilogue + output projection overlap the later
    # halves' score/PV work, shrinking the serial tail.
    phase_b(1, [(0, 0, 512), (1, 0, 512), (2, 0, 512), (3, 0, 512)])
    # drain whatever filler work remains (trailing C chunks; emit_c's
    # ensure_epi emits the final chunk's divide before its first C quantum)
    while filler:
        filler.popleft()[1]()


def get_nc():
    if "nc" not in _CACHE:
        _CACHE["nc"] = _build_nc()
    return _CACHE["nc"]


def prep_inputs(q, k, v, mask, Wq, Wk, Wv, Wo):
    """Host-side shard prep: transposes, fp16 casts, per-core weight slices."""
    q = np.asarray(q, dtype=np.float32).reshape(BS, D)
    k = np.asarray(k, dtype=np.float32).reshape(BS, D)
    v = np.asarray(v, dtype=np.float32).reshape(BS, D)
    mask = np.asarray(mask)
    Wq, Wk, Wv, Wo = (np.asarray(w, dtype=np.float32) for w in (Wq, Wk, Wv, Wo))

    xqT = np.ascontiguousarray(q.T).astype(np.float16)
    xkT = np.ascontiguousarray(k.T).astype(np.float16)
    xvT = np.ascontiguousarray(v.T).astype(np.float16)

    pb = np.where(mask == 0, np.float32(-1e9), np.float32(0.0)).astype(np.float32)
    # [B, S] -> [128, B, S//128]  (partition = j % 128, col = key tile)
    pad_bias = np.ascontiguousarray(pb.reshape(B, S // 128, 128).transpose(2, 0, 1))

    # [128, H_LOC, 128] lower-triangle mask for the diagonal sub-block:
    # within the block, token col i is live for key row p iff i >= p.
    p_idx = np.arange(128)[:, None]
    i_idx = np.arange(128)[None, :]
    cm = (i_idx >= p_idx).astype(np.float16)  # [128, 128]
    cmask = np.ascontiguousarray(
        np.broadcast_to(cm[:, None, :], (128, H_LOC, 128))
    )

    def wslice(Wmat, c):
        ws = Wmat[c * M_LOC : (c + 1) * M_LOC, :]  # [128 out, 1024 in]
        # -> [p(=d%128), kk(=d//128), c2]
        return np.ascontiguousarray(
            ws.T.reshape(8, 128, M_LOC).transpose(1, 0, 2)
        ).astype(np.float16)

    cmask_flat = cmask.reshape(128, H_LOC * 128)
    in_maps = []
    for c in range(N_CORES):
        woT_c = np.ascontiguousarray(Wo[:, c * M_LOC : (c + 1) * M_LOC].T).astype(
            np.float16
        )
        wqkv = np.ascontiguousarray(
            np.stack([wslice(Wq, c), wslice(Wk, c), wslice(Wv, c)], axis=1)
        )
        wo_cm = np.ascontiguousarray(
            np.concatenate([woT_c, cmask_flat], axis=1)
        )
        in_maps.append(
            {
                "xqT": xqT,
                "xkT": xkT,
                "xvT": xvT,
                "wqkv": wqkv,
                "wo_cm": wo_cm,
                "pad_bias": pad_bias,
            }
        )
    return in_maps


def gather_output(results):
    acc = np.zeros((BS, D), dtype=np.float32)
    for r in results:
        acc += r["y_partial"].astype(np.float32)
    return acc.reshape(B, S, D)


def kernel(q, k, v, mask, Wq, Wk, Wv, Wo):
    nc = get_nc()
    in_maps = prep_inputs(q, k, v, mask, Wq, Wk, Wv, Wo)
    res = run_bass_kernel_spmd(nc, in_maps, core_ids=list(range(N_CORES)))
    return gather_output(res.results)
